# revision 23
# baseline (speedup 1.0000x reference)
"""Trainium2 Bass kernel for single-head MHA (B=32, G=1024, D=256),
data-parallel over batch across 8 NeuronCores.

v4: host-side layout prep. The host pre-casts the (compacted) activations
to bf16 and ships BOTH layouts -- V-layout tiles ([128, 4x256] per half,
partition = token-within-tile) and the transposed dT ([D, G] per batch) --
so the device does no casts and no PE transposes. Loads split across the
two HWDGE rings (sync + scalar); stores go on the SWDGE ring (gpsimd).
All GEMMs (KT' fold, S, PV, F, l) and the softmax stay on device.

Math per batch (compacted keys first, nk = nkt*128):
  KT'  = ntT^T @ dT[:, :nk]       [D, nk]  bf16 (ntT = Wk^T Wq rows)
  ST   = KT'_kt^T @ dT  (= S^T)   [128, G] f32 per key tile kt
  PT   = exp(NORM*ST + bias_k)    bias_k = -100*mask[k] per-partition
  HT_h += V_kt^T @ PT_h           [D, 512] per half h; V = data bf16
  l    = ones^T @ tree(PT)        [1, G]
  F    = HT^T @ PTO               [G, D]   PTO = bf16(Wv^T Wo^T)
  out  = F * (1/l)[q] + b_out     (scalar_tensor_tensor on DVE)

PSUM: apool (KT') 2 + ps_s (S, double-buffered) 4 + ps_h (H/F/l) 2 = 8.
"""

import math

import numpy as np

import concourse.bass as bass
import concourse.mybir as mybir
import concourse.tile as tile
from concourse import bacc

N_CORES = 8
B = 32
G = 1024
D = 256
BPC = B // N_CORES
TOK = BPC * G
NORM = 1.0 / math.sqrt(D)
MASK_BIAS = -100.0
KD = G // 128                # 8 query tiles per batch
NKT = 5                      # key tiles computed (after compaction)
DT_CH = D // 128             # 2 chunks of the feature dim

F32 = mybir.dt.float32
I32 = mybir.dt.int32
BF16 = mybir.dt.bfloat16


def build_program(nkt: int = NKT, bpc: int = BPC, reps: int = 1,
                  enable_asserts: bool = False, skip: frozenset = frozenset(),
                  unroll: int = 4):
    assert 1 <= nkt <= KD
    nc = bacc.Bacc("TRN2", target_bir_lowering=False, debug=False,
                   enable_asserts=enable_asserts)

    tok = bpc * G
    # host-precast bf16 activations, two layouts (see kernel() below)
    dv_d = nc.dram_tensor("datav", [bpc * 2 * 128, G], BF16,
                          kind="ExternalInput").ap()
    dt_d = nc.dram_tensor("datat", [bpc * D, G], BF16,
                          kind="ExternalInput").ap()
    mask_d = nc.dram_tensor("mask", [bpc, G], I32, kind="ExternalInput").ap()
    wq_d = nc.dram_tensor("w_query", [D, D], F32, kind="ExternalInput").ap()
    wk_d = nc.dram_tensor("w_key", [D, D], F32, kind="ExternalInput").ap()
    wv_d = nc.dram_tensor("w_val", [D, D], F32, kind="ExternalInput").ap()
    wo_d = nc.dram_tensor("w_out", [D, D], F32, kind="ExternalInput").ap()
    b_d = nc.dram_tensor("b_out", [D], F32, kind="ExternalInput").ap()
    out_d = nc.dram_tensor("out", [tok, D], BF16, kind="ExternalOutput").ap()

    from contextlib import ExitStack
    with tile.TileContext(nc) as tc, ExitStack() as ctx:
        _body(ctx, tc, out_d, dv_d, dt_d, mask_d, wq_d, wk_d, wv_d, wo_d, b_d,
              nkt, bpc, reps, skip, unroll)

    nc.compile()
    return nc


def _body(ctx, tc, out_d, dv_d, dt_d, mask_d, wq_d, wk_d, wv_d, wo_d, b_d,
          nkt, bpc, reps, skip=frozenset(), unroll_req=4):
    nc = tc.nc
    from concourse.masks import make_identity

    nk = nkt * 128           # compacted key count (cols of KT')

    const = ctx.enter_context(tc.tile_pool(name="const", bufs=1))
    wpool = ctx.enter_context(tc.tile_pool(name="wpool", bufs=1))
    dn16_p = ctx.enter_context(tc.tile_pool(name="dn16", bufs=6))
    dT_p = ctx.enter_context(tc.tile_pool(name="dT", bufs=6))
    kt_p = ctx.enter_context(tc.tile_pool(name="ktp", bufs=6))
    pt_p = ctx.enter_context(tc.tile_pool(name="pt", bufs=3))
    ht_p = ctx.enter_context(tc.tile_pool(name="ht", bufs=3))
    out_p = ctx.enter_context(tc.tile_pool(name="outp", bufs=3))
    misc_p = ctx.enter_context(tc.tile_pool(name="misc", bufs=3))

    # PSUM: apool 2 + ps_s 2x2 + ps_h 2 = 8 banks
    apool = ctx.enter_context(tc.tile_pool(name="apool", bufs=2, space="PSUM"))
    ps_s = ctx.enter_context(tc.tile_pool(name="ps_s", bufs=2, space="PSUM"))
    ps_h = ctx.enter_context(tc.tile_pool(name="ps_h", bufs=2, space="PSUM"))

    # ---- constants ----------------------------------------------------------
    ident_f = const.tile([128, 128], F32, tag="identf")
    make_identity(nc, ident_f)

    ones_f32 = const.tile([128, 1], F32, tag="ones_f32")
    nc.vector.memset(ones_f32, 1.0)
    ones = const.tile([128, 1], BF16, tag="ones")
    nc.vector.tensor_copy(ones, ones_f32)

    # warm the exp table set before the first attention tile
    act_warm = const.tile([128, 1], F32, tag="act_warm")
    nc.scalar.activation(out=act_warm, in_=ones_f32,
                         func=mybir.ActivationFunctionType.Exp)

    bias_rep = const.tile([128, D], F32, tag="bias_rep")
    b_bcast = bass.AP(tensor=b_d.tensor, offset=b_d.offset,
                      ap=[[0, 128]] + list(b_d.ap))
    nc.gpsimd.dma_start(out=bias_rep, in_=b_bcast)

    # ---- weight prep: ntT = bf16(Wk^T Wq), PTO = bf16(Wv^T Wo^T) ------------
    wnat = {}
    for name, w_d in (("q", wq_d), ("k", wk_d), ("v", wv_d), ("o", wo_d)):
        ts = []
        for r in range(DT_CH):
            t = wpool.tile([128, D], F32, tag=f"wnat_{name}{r}",
                           name=f"wnat_{name}{r}")
            nc.scalar.dma_start(out=t, in_=w_d[r * 128:(r + 1) * 128, :])
            ts.append(t)
        wnat[name] = ts

    woT = []
    for c in range(DT_CH):
        wt_c = wpool.tile([128, D], F32, tag=f"woT{c}", name=f"woT{c}")
        for r in range(DT_CH):
            ps = ps_h.tile([128, 512], F32, tag="ps_h", name=f"psw{c}{r}")
            nc.tensor.transpose(ps[:, :128],
                                wnat["o"][r][:, c * 128:(c + 1) * 128], ident_f)
            nc.scalar.copy(wt_c[:, r * 128:(r + 1) * 128], ps[:, :128])
        woT.append(wt_c)

    # ntT rows m of (Wk^T Wq): lhsT for the key-side fold
    ntT = []
    for jt in range(DT_CH):
        ps = ps_s.tile([128, 1024], F32, tag="ps_s", name=f"psnt{jt}")
        for dc in range(DT_CH):
            nc.tensor.matmul(
                ps[:, :D],
                wnat["k"][dc][:, jt * 128:(jt + 1) * 128],
                wnat["q"][dc],
                start=(dc == 0), stop=(dc == DT_CH - 1))
        t = wpool.tile([128, D], BF16, tag=f"nt{jt}", name=f"nt{jt}")
        nc.scalar.copy(t, ps[:, :D])
        ntT.append(t)

    pto = []
    for dtile in range(DT_CH):
        ps = ps_s.tile([128, 1024], F32, tag="ps_s", name=f"pspt{dtile}")
        for mc in range(DT_CH):
            nc.tensor.matmul(
                ps[:, :D],
                wnat["v"][mc][:, dtile * 128:(dtile + 1) * 128],
                woT[mc],
                start=(mc == 0), stop=(mc == DT_CH - 1))
        t = wpool.tile([128, D], BF16, tag=f"pto{dtile}", name=f"pto{dtile}")
        nc.scalar.copy(t, ps[:, :D])
        pto.append(t)

    state = {}
    _seq = [0]   # unique name suffix: the wrap re-emits stage A for batch 0

    def a_load_dma(b):
        """Mask + both-layout data DMAs; no engine compute."""
        _seq[0] += 1
        u = f"{b}_{_seq[0]}"
        mb8 = misc_p.tile([128, nkt], I32, tag="mb8", name=f"mb8_{u}")
        msrc = bass.AP(tensor=mask_d.tensor, offset=mask_d.offset + b * G,
                       ap=[[1, 128], [128, nkt]])
        nc.sync.dma_start(out=mb8, in_=msrc)
        dn16, dT = [], []
        for h in range(2):
            vtiles = min(nkt - 4 * h, 4)   # key tiles resident in this half
            if vtiles > 0:
                dv = dn16_p.tile([128, vtiles * D], BF16, tag="dn16",
                                 name=f"dv_{u}_{h}")
                nc.scalar.dma_start(
                    out=dv,
                    in_=dv_d[(b * 2 + h) * 128:(b * 2 + h + 1) * 128,
                             :vtiles * D])
                dn16.append(dv)
            dc = dT_p.tile([128, G], BF16, tag="dT", name=f"dT_{u}_{h}")
            nc.sync.dma_start(
                out=dc, in_=dt_d[(b * 2 + h) * 128:(b * 2 + h + 1) * 128, :])
            dT.append(dc)
        mbT = misc_p.tile([128, nkt], F32, tag="mbT", name=f"mbT_{u}")
        nc.vector.tensor_scalar_mul(mbT, mb8, MASK_BIAS)
        state[b] = {"u": u, "V": dn16, "dT": dT, "mbT": mbT}

    def _vslice(st, g, c):
        """[128,128] bf16 slice of V for global 128-token tile g, chunk c."""
        return st["V"][g // 4][:, (g % 4) * 256 + c * 128:
                               (g % 4) * 256 + (c + 1) * 128]

    def _kt_splits():
        splits = [(0, min(nk, 512))]
        if nk > 512:
            splits.append((512, nk - 512))
        return splits

    def a_kt_split(b, it, si):
        """KT' chunk it, split si: fold (Wk^T Wq) into the compacted keys."""
        st = state[b]
        u = st["u"]
        if it == 0 and si == 0:
            st["kt16"] = [kt_p.tile([128, nk], BF16, tag="kt16",
                                    name=f"kt16_{u}_{i}") for i in range(DT_CH)]
        off, width = _kt_splits()[si]
        pk = apool.tile([128, width], F32, tag="apool",
                        name=f"pkt_{u}_{it}_{off}")
        for mc in range(DT_CH):
            nc.tensor.matmul(
                pk,
                ntT[mc][:, it * 128:(it + 1) * 128],
                st["dT"][mc][:, off:off + width],
                start=(mc == 0), stop=(mc == DT_CH - 1))
        nc.vector.tensor_copy(st["kt16"][it][:, off:off + width], pk)

    def a_kt_all(b):
        for it in range(DT_CH):
            for si in range(len(_kt_splits())):
                a_kt_split(b, it, si)

    def b_open(b):
        st = state[b]
        u = st["u"]
        st["pt"] = pt_p.tile([128, nkt * G], BF16, tag="pt", name=f"pt_{u}")
        st["HT"] = [ht_p.tile([128, G], BF16, tag=f"hT{i}", name=f"hT_{u}_{i}")
                    for i in range(DT_CH)]
        st["lrow"] = misc_p.tile([1, G], F32, tag="l_row", name=f"lrow_{u}")

    def emit_s(b, kt_i):
        st = state[b]
        pss = ps_s.tile([128, 1024], F32, tag="ps_s",
                        name=f"pss_{st['u']}_{kt_i}")
        for ic in range(DT_CH):
            for h in range(2):
                nc.tensor.matmul(
                    pss[:, h * 512:(h + 1) * 512],
                    st["kt16"][ic][:, kt_i * 128:(kt_i + 1) * 128],
                    st["dT"][ic][:, h * 512:(h + 1) * 512],
                    start=(ic == 0), stop=(ic == DT_CH - 1))
        if "exp" in skip:   # timing ablation: keep topology, drop ACT cost
            nc.scalar.copy(st["pt"][:, kt_i * G:kt_i * G + 1], pss[:, :1])
        else:
            nc.scalar.activation(
                out=st["pt"][:, kt_i * G:(kt_i + 1) * G], in_=pss,
                func=mybir.ActivationFunctionType.Exp,
                bias=st["mbT"][:, kt_i:kt_i + 1], scale=NORM)

    def pv_pair(b):
        st = state[b]
        psH0 = [ps_h.tile([128, 512], F32, tag="ps_h",
                          name=f"psH_{st['u']}_0_{i}") for i in range(DT_CH)]
        psH1 = [apool.tile([128, 512], F32, tag="apool",
                           name=f"psH_{st['u']}_1_{i}") for i in range(DT_CH)]
        def emit_pv(kt_i):
            for h, psH in ((0, psH0), (1, psH1)):
                for dt_i in range(DT_CH):
                    nc.tensor.matmul(
                        psH[dt_i],
                        _vslice(st, kt_i, dt_i),
                        st["pt"][:, kt_i * G + h * 512:
                                 kt_i * G + (h + 1) * 512],
                        start=(kt_i == 0), stop=(kt_i == nkt - 1))
        return psH0, psH1, emit_pv

    # l tree on DVE: bf16 adds pair-reduce the first nkt-1 PT tiles; the
    # final add is gated on the last exp and emitted separately.
    def _ladd(b, li, t0, o0, t1, o1):
        sm = misc_p.tile([128, G], BF16, tag=f"lsum{li}",
                         name=f"lsum_{state[b]['u']}_{li}")
        nc.vector.tensor_tensor(out=sm, in0=t0[:, o0:o0 + G],
                                in1=t1[:, o1:o1 + G], op=mybir.AluOpType.add)
        return sm

    def emit_lsums_partial(b):
        st = state[b]
        srcs = [(st["pt"], kt_i * G) for kt_i in range(nkt - 1)]
        li = 0
        while len(srcs) > 1:
            dsts = []
            for i in range(0, len(srcs) - 1, 2):
                (t0, o0), (t1, o1) = srcs[i], srcs[i + 1]
                dsts.append((_ladd(b, li, t0, o0, t1, o1), 0))
                li += 1
            if len(srcs) % 2:
                dsts.append(srcs[-1])
            srcs = dsts
        st["lpart"] = srcs[0]
        st["lnext"] = li

    def emit_lsum_final(b):
        st = state[b]
        t0, o0 = st["lpart"]
        if nkt > 1:
            st["ltot"] = (_ladd(b, st["lnext"], t0, o0,
                                st["pt"], (nkt - 1) * G), 0)
        else:
            st["ltot"] = (st["pt"], 0)

    def emit_l(b, h):
        st = state[b]
        t, off = st["ltot"]
        psl = ps_h.tile([1, 512], F32, tag="ps_h",
                        name=f"psl_{st['u']}_{h}")
        nc.tensor.matmul(psl, ones, t[:, off + h * 512:off + (h + 1) * 512],
                         start=True, stop=True)
        # drain to the SBUF row on ACT
        nc.scalar.copy(st["lrow"][:, h * 512:(h + 1) * 512], psl)

    def b_inv(b):
        st = state[b]
        u = st["u"]
        # relayout the [1, G] row into [KD, 128] (linear copy), then one
        # PE transpose gives the [128, KD] per-partition layout
        lb8 = misc_p.tile([KD, 128], F32, tag="lb8", name=f"lb8_{u}")
        nc.scalar.dma_start(out=lb8, in_=bass.AP(
            tensor=st["lrow"].tensor, offset=st["lrow"].offset,
            ap=[list(st["lrow"].ap[0]), [128, KD], [1, 128]]))
        psinv = ps_h.tile([128, KD], F32, tag="ps_h", name=f"psinv_{u}")
        nc.tensor.transpose(psinv, lb8, ident_f[:KD, :KD])
        invl = misc_p.tile([128, KD], F32, tag="invl", name=f"invl_{u}")
        nc.vector.reciprocal(invl, psinv)
        st["invl"] = invl

    def stage_f(b, p_i):
        """Final projection for query tiles 2*p_i, 2*p_i+1."""
        st = state[b]
        ps = ps_h.tile([128, 512], F32, tag="ps_h",
                       name=f"psf_{st['u']}_{p_i}")
        for j in range(2):
            qt_i = p_i * 2 + j
            for dt_i in range(DT_CH):
                nc.tensor.matmul(
                    ps[:, j * D:(j + 1) * D],
                    st["HT"][dt_i][:, qt_i * 128:(qt_i + 1) * 128],
                    pto[dt_i],
                    start=(dt_i == 0), stop=(dt_i == DT_CH - 1))
        return ps

    def stage_f_drain(b, p_i, ps):
        st = state[b]
        for j in range(2):
            qt_i = p_i * 2 + j
            nc.vector.scalar_tensor_tensor(
                out=st["ot"][:, qt_i * D:(qt_i + 1) * D],
                in0=ps[:, j * D:(j + 1) * D],
                scalar=st["invl"][:, qt_i:qt_i + 1], in1=bias_rep,
                op0=mybir.AluOpType.mult, op1=mybir.AluOpType.add)

    def out_dma(b, h):
        st = state[b]
        row0 = b * G
        dst = bass.AP(tensor=out_d.tensor,
                      offset=out_d.offset + (row0 + h * 512) * D,
                      ap=[[D, 128], [128 * D, 4], [1, D]])
        eng = nc.sync if h == 0 else nc.gpsimd
        eng.dma_start(out=dst, in_=st["ot"][:, h * 1024:(h + 1) * 1024])

    wrap = reps > 1
    proto = {}

    def a_reuse(b):
        """Timing ablation: reuse batch 0's stage-A outputs for batch b."""
        _seq[0] += 1
        state[b] = {"u": f"{b}_{_seq[0]}", "V": proto["V"],
                    "dT": proto["dT"], "kt16": proto["kt16"],
                    "mbT": proto["mbT"]}

    def batch_body(b):
        nxt = b + 1 if b + 1 < bpc else (0 if wrap else None)
        if "compute" in skip:   # DMA-only microbench: loads + stores
            st = state[b]
            st["ot"] = out_p.tile([128, KD * D], BF16, tag="outp",
                                  name=f"ot_{st['u']}")
            nc.vector.memset(st["ot"], 0.0)
            if nxt is not None:
                a_load_dma(nxt)
            out_dma(b, 0)
            out_dma(b, 1)
            del state[b]
            return
        skip_a = "stagea" in skip
        st = state[b]
        b_open(b)
        st["ot"] = out_p.tile([128, KD * D], BF16, tag="outp",
                              name=f"ot_{st['u']}")
        if nxt is not None and not skip_a:
            a_load_dma(nxt)       # DMAs land while this batch computes

        psH0, psH1, emit_pv = pv_pair(b)
        emit_s(b, 0)
        for kt_i in range(1, nkt):
            emit_s(b, kt_i)
            if "pv" not in skip:
                emit_pv(kt_i - 1)
            if kt_i == 3 and "l" not in skip:
                emit_lsums_partial(b)
        if nkt <= 3 and "l" not in skip:
            emit_lsums_partial(b)
        if "pv" not in skip:
            emit_pv(nkt - 1)
        if "l" not in skip:
            emit_lsum_final(b)

        if "pv" not in skip:
            nc.vector.tensor_copy(st["HT"][0][:, 0:512], psH0[0])
            nc.scalar.copy(st["HT"][1][:, 0:512], psH0[1])
            nc.vector.tensor_copy(st["HT"][0][:, 512:1024], psH1[0])
            nc.scalar.copy(st["HT"][1][:, 512:1024], psH1[1])
        if "l" not in skip:
            emit_l(b, 0)
            emit_l(b, 1)
            b_inv(b)

        # final projection, interleaved with next batch's KT' fold so the
        # PSUM drains hide
        nsplits = len(_kt_splits()) if (nxt is not None and not skip_a) else 0

        def kt_step(i):
            if i < 2 * nsplits:
                a_kt_split(nxt, i // nsplits, i % nsplits)

        # next batch's KT' fold first -- its DVE drains overlap the F
        # matmuls, so the next S-loop can start during F2/F3
        for i in range(2 * nsplits):
            kt_step(i)
        if "f" not in skip:
            psf0 = stage_f(b, 0)
            psf1 = stage_f(b, 1)
            stage_f_drain(b, 0, psf0)
            psf2 = stage_f(b, 2)
            stage_f_drain(b, 1, psf1)
            if "store" not in skip:
                out_dma(b, 0)
            psf3 = stage_f(b, 3)
            stage_f_drain(b, 2, psf2)
            stage_f_drain(b, 3, psf3)
            if "store" not in skip:
                out_dma(b, 1)
        if nxt is not None and skip_a:
            a_reuse(nxt)
        del state[b]

    # prologue: stage A of batch 0 runs once, outside the loop
    a_load_dma(0)
    a_kt_all(0)
    proto.update({k: state[0][k] for k in ("V", "dT", "kt16", "mbT")})

    # the For_i back-edge costs a ~7us all-engine barrier; unroll several
    # full passes into one loop body to amortize it in reps benchmarking
    unroll = unroll_req if (reps > 1 and reps % unroll_req == 0) else 1
    if reps > 1:
        loop_cm = tc.For_i(0, reps // unroll, 1)
        loop_cm.__enter__()

    for _ in range(unroll):
        for b in range(bpc):
            batch_body(b)

    if reps > 1:
        loop_cm.__exit__(None, None, None)


# ---------------------------------------------------------------------------
# Host side: compaction + layout prep + cached jax.jit(shard_map) runner.
def compact(data, mask):
    """Per-batch stable-sort of rows so unmasked keys come first."""
    nb = mask.shape[0]
    datac = np.empty_like(data)
    maskc = np.empty_like(mask)
    perms = np.empty((nb, G), np.int64)
    for b in range(nb):
        p = np.argsort(mask[b], kind="stable")
        datac[b * G:(b + 1) * G] = data[b * G:(b + 1) * G][p]
        maskc[b] = mask[b][p]
        perms[b] = p
    return datac, maskc, perms


_RUNNER_CACHE = {}


def _make_runner(nkt):
    import jax
    from jax.experimental.shard_map import shard_map
    from jax.sharding import Mesh, NamedSharding, PartitionSpec

    from concourse.bass2jax import (
        _bass_exec_p,
        install_neuronx_cc_hook,
        partition_id_tensor,
    )

    nc = build_program(nkt)
    install_neuronx_cc_hook()
    assert nc.dbg_addr is None
    partition_name = (nc.partition_id_tensor.name
                      if nc.partition_id_tensor else None)

    in_names, out_names, out_avals, zero_outs = [], [], [], []
    for alloc in nc.m.functions[0].allocations:
        if not isinstance(alloc, mybir.MemoryLocationSet):
            continue
        name = alloc.memorylocations[0].name
        if alloc.kind == "ExternalInput":
            if name != partition_name:
                in_names.append(name)
        elif alloc.kind == "ExternalOutput":
            shape = tuple(alloc.tensor_shape)
            dtype = mybir.dt.np(alloc.dtype)
            out_names.append(name)
            out_avals.append(jax.core.ShapedArray(shape, dtype))
            zero_outs.append(np.zeros((N_CORES * shape[0],) + shape[1:], dtype))
    n_params = len(in_names)
    all_in_names = list(in_names) + list(out_names)
    if partition_name is not None:
        all_in_names.append(partition_name)

    def _body_fn(*args):
        operands = list(args)
        if partition_name is not None:
            operands.append(partition_id_tensor())
        outs = _bass_exec_p.bind(
            *operands,
            out_avals=tuple(out_avals),
            in_names=tuple(all_in_names),
            out_names=tuple(out_names),
            lowering_input_output_aliases=(),
            sim_require_finite=False,
            sim_require_nnan=False,
            nc=nc,
        )
        return tuple(outs)

    devices = jax.devices()[:N_CORES]
    mesh = Mesh(np.asarray(devices), ("core",))
    in_specs = (PartitionSpec("core"),) * (n_params + len(out_names))
    out_specs = (PartitionSpec("core"),) * len(out_names)
    sharded = jax.jit(
        shard_map(_body_fn, mesh=mesh, in_specs=in_specs, out_specs=out_specs,
                  check_rep=False),
        keep_unused=True,
    )
    sharding = NamedSharding(mesh, PartitionSpec("core"))
    dev_zeros = [jax.device_put(z, sharding) for z in zero_outs]
    return {
        "nc": nc, "fn": sharded, "in_names": in_names,
        "out_names": out_names, "sharding": sharding, "dev_zeros": dev_zeros,
    }


def get_runner(nkt=NKT):
    if nkt not in _RUNNER_CACHE:
        _RUNNER_CACHE[nkt] = _make_runner(nkt)
    return _RUNNER_CACHE[nkt]


def _concat_inputs(datac, maskc, wq, wk, wv, wo, b):
    """Per-core shards concatenated on axis 0, keyed by dram tensor name.

    Builds both bf16 layouts of the compacted activations:
      datav: per (batch, half) a [128, 4*256] tile, partition = token%128
      datat: per batch the [D, G] transpose
    """
    import ml_dtypes
    bf16 = ml_dtypes.bfloat16
    d16 = datac.astype(bf16)
    # [B, half, token-in-tile(128), tile(4), feat] -> rows (b, h, p)
    dv = (d16.reshape(B, 2, 4, 128, D).transpose(0, 1, 3, 2, 4)
          .reshape(B * 2 * 128, 4 * D))
    dt = (d16.reshape(B, G, D).transpose(0, 2, 1).reshape(B * D, G))
    return {
        "datav": np.ascontiguousarray(dv),
        "datat": np.ascontiguousarray(dt),
        "mask": maskc,
        "w_query": np.concatenate([wq] * N_CORES, axis=0),
        "w_key": np.concatenate([wk] * N_CORES, axis=0),
        "w_val": np.concatenate([wv] * N_CORES, axis=0),
        "w_out": np.concatenate([wo] * N_CORES, axis=0),
        "b_out": np.concatenate([b] * N_CORES, axis=0),
    }


def kernel(data, mask, graph_size, evaluate, W_query, W_key, W_val, W_out,
           b_out, **_ignored):
    data = np.ascontiguousarray(np.asarray(data, dtype=np.float32))
    mask = np.ascontiguousarray(np.asarray(mask, dtype=np.int32))
    wq = np.ascontiguousarray(np.asarray(W_query, dtype=np.float32))
    wk = np.ascontiguousarray(np.asarray(W_key, dtype=np.float32))
    wv = np.ascontiguousarray(np.asarray(W_val, dtype=np.float32))
    wo = np.ascontiguousarray(np.asarray(W_out, dtype=np.float32))
    b = np.ascontiguousarray(np.asarray(b_out, dtype=np.float32))

    datac, maskc, perms = compact(data, mask)
    nk_max = int((G - maskc.sum(axis=1)).max())
    nkt = max(NKT, -(-nk_max // 128))   # ceil; >=NKT so the cached program wins

    r = get_runner(nkt)
    cat = _concat_inputs(datac, maskc, wq, wk, wv, wo, b)
    args = [cat[n] for n in r["in_names"]] + list(r["dev_zeros"])
    outs = r["fn"](*args)
    outc = np.asarray(outs[r["out_names"].index("out")]).astype(np.float32)

    out = np.empty_like(outc)
    rows = (perms + (np.arange(B)[:, None] * G)).reshape(-1)
    out[rows] = outc
    return out
